# revision 48
# baseline (speedup 1.0000x reference)
"""Trainium2 Bass/Tile kernel: supervised contrastive loss (N=8192, D=256).

Reference math (jax): r = x / max(||x||, 1e-12); sim = r @ r.T;
  neg_ij = (label_i != label_j); den_i = sum_j exp(sim_ij * neg_ij / 0.1) + 1
  loss = mean_i log(den_i + 1e-8)
Since exp(sim_ij * neg_ij / T) == 1 for every same-label pair (incl. the
diagonal), den_i = sum_{j: l_j != l_i} exp(sim_ij/T) + count_same_i + 1.

The kernel is Activation-engine bound: the N^2/8 = 8.4M exp evaluations per
core cost 32 x (2048 * 0.83ns + overhead) ~ 65us on ACT and nothing else
comes close.  The design therefore strips every other op off ACT:

  * Rows are SORTED BY LABEL on the host (the loss is a mean over rows, so
    any permutation is exact) and each core's input is additionally rotated
    so its own 1024 rows sit at columns [0, 1024).  Same-label pairs then
    live in a narrow diagonal band, so the one-hot mask matmul (which
    costs 2x the fp8 similarity matmul per chunk) is only emitted for the
    1-2 512-col chunks per row-tile that intersect the band - computed
    from the actual labels at runtime and compiled per band pattern.
    PE time drops ~51us -> ~18us.
  * Inverse row norms are computed COMPACTLY instead of replicated:
    a row-major copy of x is reduced with fused square+rowsum
    (scalar_tensor_tensor, split across DVE and the otherwise-idle GPSIMD)
    into [128, 16] per group, passed through ln/exp rsqrt on ACT at
    free-size 16 (~0.2us vs the 15.4us a [128, 2048] replicated ln/exp
    pass costs), reshaped to a partition-0 row by a tiny SBUF->SBUF DMA,
    broadcast to 128 partitions by gpsimd.partition_broadcast, and fused
    into the bf16->fp8 cast of the matmul operand.
  * exp/ln share one activation-table set (natural_log_exp_and_others,
    forced via the table map) so ACT loads tables exactly once.
  * Main loop per 2048-col group: 8 row-tiles x (4 fp8 DoubleRow matmuls
    + band mask matmuls) into [128, 2048] PSUM, one ACT exp (scale=10)
    with fused accum_out row-sum.  Norm chain for group g+1 is emitted
    after main(g) row-tile 1 so the tiny ln/exp never stalls ACT.
  * count_same_i is restored exactly via a one-hot @ histogram matmul;
    den = rowsum + count + 1; ln; per-core partial sum via fp32 matmul.
    Host sums the 8 partials and divides by N ("all-reduce").
"""

import numpy as np
import ml_dtypes

N = 8192
D = 256
NCORES = 8
OWN = N // NCORES          # 1024 rows per core
ISCALE = 10.0              # 1 / temperature
NEGB = -5.0                # mask bias: exp(10*(sim-5)) ~ 0
CHUNK = 512                # matmul free-dim tile
GRP = 2048                 # column group width (4 PSUM banks)
NG = N // GRP              # 4 column groups
MT = OWN // 128            # 8 row tiles per core
RT = GRP // 128            # 16 sumsq row-tiles per group
NSQ_DVE = 10               # sumsq split: 10 tiles on DVE, 6 on GPSIMD

_CACHE = {}


def _build(bands):
    """bands: per row-tile m, tuple of global 512-chunk indices that get the
    one-hot mask matmul (same for every core thanks to the rotation)."""
    import concourse.bass as bass
    import concourse.tile as tile
    import concourse.bacc as bacc_mod
    from concourse import bacc, mybir
    from contextlib import ExitStack

    f32 = mybir.dt.float32
    bf16 = mybir.dt.bfloat16
    f8 = mybir.dt.float8e4
    Alu = mybir.AluOpType
    Act = mybir.ActivationFunctionType
    AX = mybir.AxisListType.X

    # Force Exp and Ln to resolve to the one table set that holds both, so
    # interleaved ln/exp never reloads ACT tables.
    orig_gat = bacc_mod.get_activation_tables

    def gat_shared(arch):
        tabs = orig_gat(arch)
        for name, fns in tabs.items():
            if name != "natural_log_exp_and_others":
                fns.discard(Act.Exp)
                fns.discard(Act.Ln)
        return tabs

    bacc_mod.get_activation_tables = gat_shared
    try:
        nc = bacc.Bacc("TRN2", target_bir_lowering=False, debug=False,
                       num_devices=NCORES)

        xr_d = nc.dram_tensor("xr", [NG, 128, RT, D], bf16,
                              kind="ExternalInput")
        xt_d = nc.dram_tensor("xt", [D, N], bf16, kind="ExternalInput")
        oh_d = nc.dram_tensor("oh", [128, N], bf16, kind="ExternalInput")
        ohp_d = nc.dram_tensor("ohp", [128, OWN], bf16, kind="ExternalInput")
        out_d = nc.dram_tensor("out", [1, 1], f32, kind="ExternalOutput")

        onesf_d = nc.inline_tensor(np.ones((128, 1), dtype=np.float32),
                                   "onesf_c")
        ones128_d = nc.inline_tensor(
            np.ones((128, 128), dtype=ml_dtypes.bfloat16), "ones128_c")

        with tile.TileContext(nc) as tc:
            with ExitStack() as top:
                persist = top.enter_context(
                    tc.tile_pool(name="persist", bufs=1))
                work = top.enter_context(tc.tile_pool(name="work", bufs=2))
                psum = top.enter_context(
                    tc.tile_pool(name="psum", bufs=2, space="PSUM"))

                XR = persist.tile([128, NG, RT, D], bf16)
                XT = persist.tile([128, 2, N], bf16)
                RF = persist.tile([128, 2, N], f8)
                OH = persist.tile([128, N], bf16)
                OHP = persist.tile([128, OWN], bf16)
                OHB = persist.tile([128, OWN], bf16)
                S = persist.tile([128, NG, RT], f32)
                INVB = persist.tile([128, NG, RT], bf16)
                IVR = persist.tile([1, N], bf16)
                DP = persist.tile([128, MT * NG + 12], f32)
                H4 = persist.tile([128, NG], f32)
                CNT = persist.tile([128, MT], f32)
                DEN = persist.tile([128, MT], f32)
                LV = persist.tile([128, MT], f32)
                LS = persist.tile([128, 1], f32)
                hist_f = persist.tile([128, 1], f32)
                hist_b = persist.tile([128, 1], bf16)
                onesf_sb = persist.tile([128, 1], f32)
                ones128_sb = persist.tile([128, 128], bf16)
                MAGIC = persist.tile([128, RT], f32)
                outsb = persist.tile([1, 1], f32)
                nc.vector.memset(MAGIC.bitcast(mybir.dt.int32), 0x5F3759DF)

                def dma_grp(g, with_xr=True):
                    gs, ge = g * GRP, (g + 1) * GRP
                    nc.sync.dma_start(XT[:, 0, gs:ge], xt_d[0:128, gs:ge])
                    nc.sync.dma_start(XT[:, 1, gs:ge], xt_d[128:256, gs:ge])
                    if with_xr:
                        nc.sync.dma_start(XR[:, g], xr_d[g])

                def dma_g0_sliver(s):
                    sl = slice(s * 1024, (s + 1) * 1024)
                    nc.sync.dma_start(XT[:, 0, sl], xt_d[0:128, sl])
                    nc.sync.dma_start(XT[:, 1, sl], xt_d[128:256, sl])

                def dma_oh(g):
                    gs, ge = g * GRP, (g + 1) * GRP
                    nc.sync.dma_start(OH[:, gs:ge], oh_d[:, gs:ge])

                def norm0():
                    # group 0 startup path: replicated colsum-rsqrt in 512
                    # slivers - ACT is idle before the main loop, so the
                    # [128, 512] ln/exp passes are free, and the chain skips
                    # the reshape-DMA + partition_broadcast latency.  Casts
                    # split DVE (h=0) / GPSIMD (h=1) to shorten the serial
                    # DVE chain.
                    # stage-ordered emission: per-engine queues are in-order,
                    # so interleaving stages per sliver would let cast(s0)
                    # block sq(s1) at the DVE queue head and serialize the
                    # whole prefix.
                    sqs, ivrs = [], []
                    for s in range(4):
                        sl = slice(s * CHUNK, (s + 1) * CHUNK)
                        sq = work.tile([128, 2, CHUNK], bf16,
                                       tag=f"sq0_{s}")
                        nc.vector.tensor_tensor(out=sq[:, 0],
                                                in0=XT[:, 0, sl],
                                                in1=XT[:, 0, sl],
                                                op=Alu.mult)
                        nc.vector.tensor_tensor(out=sq[:, 1],
                                                in0=XT[:, 1, sl],
                                                in1=XT[:, 1, sl],
                                                op=Alu.mult)
                        sqs.append(sq)
                    for s in range(4):
                        pn = psum.tile([128, CHUNK], f32, tag="mm")
                        nc.tensor.matmul(pn, ones128_sb, sqs[s][:, 0],
                                         start=True, stop=False)
                        nc.tensor.matmul(pn, ones128_sb, sqs[s][:, 1],
                                         start=False, stop=True)
                        lnv = work.tile([128, CHUNK], f32, tag="lnv0")
                        nc.scalar.activation(lnv, pn, Act.Ln)
                        ivr = work.tile([128, CHUNK], bf16, tag=f"ivr0_{s}")
                        nc.scalar.activation(ivr, lnv, Act.Exp, scale=-0.5)
                        ivrs.append(ivr)
                    for s in range(4):
                        sl = slice(s * CHUNK, (s + 1) * CHUNK)
                        nc.vector.tensor_tensor(out=RF[:, 0, sl],
                                                in0=XT[:, 0, sl],
                                                in1=ivrs[s],
                                                op=Alu.mult)
                        nc.gpsimd.tensor_tensor(out=RF[:, 1, sl],
                                                in0=XT[:, 1, sl],
                                                in1=ivrs[s],
                                                op=Alu.mult)

                def norm(g):
                    gs, ge = g * GRP, (g + 1) * GRP
                    # fused square+rowsum, compact [128, 16] per group
                    # (DVE only: TensorScalarPtr is not in the Pool ISA)
                    for t in range(RT):
                        sq = work.tile([128, D], bf16, tag="sqv")
                        nc.vector.scalar_tensor_tensor(
                            out=sq, in0=XR[:, g, t], scalar=1.0,
                            in1=XR[:, g, t], op0=Alu.mult, op1=Alu.mult,
                            accum_out=S[:, g, t:t + 1])
                    # rsqrt on DVE (bit-hack seed + 2 Newton steps) so the
                    # ACT queue carries nothing but the main exp stream
                    i32 = mybir.dt.int32
                    y = work.tile([128, RT], f32, tag="y")
                    t1 = work.tile([128, RT], f32, tag="t1")
                    nc.vector.tensor_scalar(
                        out=y.bitcast(i32), in0=S[:, g].bitcast(i32),
                        scalar1=1, scalar2=None,
                        op0=Alu.logical_shift_right)
                    nc.vector.tensor_tensor(out=y.bitcast(i32),
                                            in0=MAGIC.bitcast(i32),
                                            in1=y.bitcast(i32),
                                            op=Alu.subtract)
                    for it in range(2):
                        last = it == 1
                        nc.vector.tensor_tensor(out=t1, in0=y, in1=y,
                                                op=Alu.mult)
                        nc.vector.tensor_tensor(out=t1, in0=t1, in1=S[:, g],
                                                op=Alu.mult)
                        nc.vector.tensor_scalar(
                            out=t1, in0=t1, scalar1=-0.5, scalar2=1.5,
                            op0=Alu.mult, op1=Alu.add)
                        nc.vector.tensor_tensor(
                            out=INVB[:, g] if last else y,
                            in0=t1, in1=y, op=Alu.mult)
                    # compact [128,16] -> row [1,2048] (col j = 16*p + t,
                    # matching the host xr layout)
                    nc.sync.dma_start(IVR[0:1, gs:ge], INVB[:, g])
                    ib = work.tile([128, GRP], bf16, tag="ib")
                    nc.gpsimd.partition_broadcast(ib, IVR[0:1, gs:ge])
                    # fused normalize + bf16->fp8 cast of the matmul
                    # operand, split DVE (h=0) / GPSIMD (h=1) so the two
                    # halves run in parallel
                    nc.vector.tensor_tensor(out=RF[:, 0, gs:ge],
                                            in0=XT[:, 0, gs:ge], in1=ib,
                                            op=Alu.mult)
                    nc.gpsimd.tensor_tensor(out=RF[:, 1, gs:ge],
                                            in0=XT[:, 1, gs:ge], in1=ib,
                                            op=Alu.mult)

                # DP slot layout: the first NSPLIT row-tiles get 4 sub-slots
                # for their group-0 tile (512-wide exps so ACT saturates
                # while norm0 slivers land, each chunk gated only by its
                # own sliver's cast).
                NSPLIT = 2

                # split tiles use 2 sub-slots (1024-wide exps) + 3 group
                # slots = 5 cols; others use 4.  All cols in a reduce range
                # are written.
                def dp_slot(m, g):
                    if m < NSPLIT:
                        return m * 5 + 1 + g
                    return NSPLIT * 5 + (m - NSPLIT) * NG + g

                def dp_range(m):
                    if m < NSPLIT:
                        return (m * 5, m * 5 + 5)
                    lo = NSPLIT * 5 + (m - NSPLIT) * NG
                    return (lo, lo + NG)

                def main_tiles(g, ms):
                    for m in ms:
                        ml = m * 128
                        ps = psum.tile([128, GRP], f32, tag="mm")
                        for s in range(GRP // CHUNK):
                            k = g * (GRP // CHUNK) + s
                            c0 = k * CHUNK
                            masked = k in bands[m]
                            nc.tensor.matmul(
                                ps[:, s * CHUNK:(s + 1) * CHUNK],
                                RF[:, :, ml:ml + 128],
                                RF[:, :, c0:c0 + CHUNK],
                                start=True, stop=not masked,
                                perf_mode=mybir.MatmulPerfMode.DoubleRow)
                            if masked:
                                nc.tensor.matmul(
                                    ps[:, s * CHUNK:(s + 1) * CHUNK],
                                    OHB[:, ml:ml + 128],
                                    OH[:, c0:c0 + CHUNK],
                                    start=False, stop=True)
                            if m < NSPLIT and g == 0 and s % 2 == 1:
                                sub = slice((s - 1) * CHUNK,
                                            (s + 1) * CHUNK)
                                nc.scalar.activation(
                                    out=ps[:, sub], in_=ps[:, sub],
                                    func=Act.Exp, scale=ISCALE,
                                    accum_out=DP[:, m * 5 + s // 2:
                                                 m * 5 + s // 2 + 1])
                        if not (m < NSPLIT and g == 0):
                            sl = dp_slot(m, g)
                            nc.scalar.activation(
                                out=ps, in_=ps, func=Act.Exp, scale=ISCALE,
                                accum_out=DP[:, sl:sl + 1])
                        if g == NG - 1:
                            lo, hi = dp_range(m)
                            nc.vector.reduce_sum(
                                DEN[:, m:m + 1], DP[:, lo:hi], axis=AX)

                def hist(g):
                    hs = work.tile([128, GRP], bf16, tag="hs")
                    nc.vector.tensor_scalar(
                        out=hs, in0=OH[:, g * GRP:(g + 1) * GRP],
                        scalar1=1.0, scalar2=None, op0=Alu.mult,
                        op1=Alu.add, accum_out=H4[:, g:g + 1])

                # ---- emission (per-engine queue order is the schedule) ----
                nc.sync.dma_start(ones128_sb, ones128_d[:])
                for s in range(2):
                    dma_g0_sliver(s)
                # PE warm-up: tiny matmul chain so the p-state ramp happens
                # before the first real colsum instead of during it
                pw = psum.tile([1, RT], f32, tag="mm")
                for w in range(16):
                    nc.tensor.matmul(pw, MAGIC[:, 0:1], MAGIC[:, 0:RT],
                                     start=True, stop=True)
                norm0()
                dma_oh(0)
                nc.sync.dma_start(OHP, ohp_d[:])
                nc.sync.dma_start(onesf_sb, onesf_d[:])
                nc.sync.dma_start(XR[:, 1], xr_d[1])
                dma_grp(1, with_xr=False)
                nc.vector.tensor_scalar(out=OHB, in0=OHP, scalar1=NEGB,
                                        scalar2=None, op0=Alu.mult)
                norm(1)            # two-group lookahead: norm(g+1) always
                dma_grp(2)
                main_tiles(0, range(0, 2))
                norm(2)
                dma_grp(3)
                main_tiles(0, range(2, MT))
                dma_oh(1)
                hist(0)
                main_tiles(1, range(0, 2))
                norm(3)
                dma_oh(2)
                main_tiles(1, range(2, MT))
                hist(1)
                main_tiles(2, range(0, 2))
                dma_oh(3)
                main_tiles(2, range(2, MT))
                hist(2)
                hist(3)
                nc.vector.reduce_sum(hist_f, H4, axis=AX)
                nc.vector.tensor_copy(hist_b, hist_f)
                # count_same via label histogram, slotted before the last
                # group so the kernel tail stays short
                psc = psum.tile([128, GRP], f32, tag="mm")
                for m in range(MT):
                    nc.tensor.matmul(psc[:, m:m + 1],
                                     OHP[:, m * 128:(m + 1) * 128],
                                     hist_b, start=True, stop=True)
                nc.vector.tensor_copy(CNT, psc[:, 0:MT])
                main_tiles(3, range(MT))

                # finale: den = rowsum + count + 1 (reference's +1e-8 is
                # below fp32 ulp at den ~ 1e4), ln, per-core partial sum
                nc.vector.scalar_tensor_tensor(
                    out=DEN, in0=DEN, scalar=1.0, in1=CNT,
                    op0=Alu.add, op1=Alu.add)
                nc.scalar.activation(LV, DEN, Act.Ln)
                nc.vector.reduce_sum(LS, LV, axis=AX)
                psf = psum.tile([1, 1], f32, tag="mm")
                nc.tensor.matmul(psf, LS, onesf_sb, start=True, stop=True)
                nc.vector.tensor_copy(outsb, psf)
                nc.sync.dma_start(out_d[:], outsb)

        nc.compile()
    finally:
        bacc_mod.get_activation_tables = orig_gat
    return nc


def _get_nc(bands=None):
    if bands is None:
        bands = _CACHE.get("last_bands")
    if bands is None:
        raise RuntimeError("call kernel() first")
    key = ("nc", bands)
    if key not in _CACHE:
        _CACHE[key] = _build(bands)
    _CACHE["last_bands"] = bands
    _CACHE["nc"] = _CACHE[key]
    return _CACHE[key]


def _prep(representations, pseudo_labels):
    """Sort rows by label; build per-core rotated inputs and the uniform
    near-diagonal band pattern."""
    x = np.asarray(representations, dtype=np.float32)
    labels = np.asarray(pseudo_labels).astype(np.int64).reshape(N)
    perm = np.argsort(labels, kind="stable")
    ls = labels[perm]
    xsb = np.ascontiguousarray(x[perm]).astype(ml_dtypes.bfloat16)
    xtb = np.ascontiguousarray(xsb.T)                      # [256, N]
    oh_s = (ls[None, :] == np.arange(128, dtype=np.int64)[:, None])
    oh_s = np.ascontiguousarray(oh_s).astype(ml_dtypes.bfloat16)

    # same-label run bounds per row (sorted order)
    grp_start = np.zeros(N, dtype=np.int64)
    grp_end = np.zeros(N, dtype=np.int64)
    starts = np.flatnonzero(np.r_[True, ls[1:] != ls[:-1]])
    ends = np.r_[starts[1:], N]
    for s, e in zip(starts, ends):
        grp_start[s:e] = s
        grp_end[s:e] = e

    # uniform band pattern: union over cores of the rotated chunk windows
    chunksets = [set() for _ in range(MT)]
    for c in range(NCORES):
        for m in range(MT):
            r0 = c * OWN + m * 128
            r1 = r0 + 127
            ws = int(grp_start[r0]) - c * OWN
            we = ws + int(grp_end[r1] - grp_start[r0])
            ws_l = ws % N
            we_l = ws_l + (we - ws)
            for k in range(ws_l // CHUNK, (we_l - 1) // CHUNK + 1):
                chunksets[m].add(k % (N // CHUNK))
    bands = tuple(tuple(sorted(s)) for s in chunksets)

    in_maps = []
    for c in range(NCORES):
        r = c * OWN
        xc = np.roll(xsb, -r, axis=0)
        xr = np.ascontiguousarray(xc.reshape(NG, 128, RT, D))
        xt = np.ascontiguousarray(np.roll(xtb, -r, axis=1))
        oh = np.ascontiguousarray(np.roll(oh_s, -r, axis=1))
        in_maps.append({
            "xr": xr,
            "xt": xt,
            "oh": oh,
            "ohp": np.ascontiguousarray(oh[:, 0:OWN]),
        })
    return in_maps, bands


def kernel(representations, pseudo_labels):
    from concourse.bass_utils import run_bass_kernel_spmd

    in_maps, bands = _prep(representations, pseudo_labels)
    nc = _get_nc(bands)
    res = run_bass_kernel_spmd(nc, in_maps, list(range(NCORES)))
    total = np.sum([np.float64(res.results[c]["out"][0, 0])
                    for c in range(NCORES)])
    return np.float32(total / N)
